# revision 43
# baseline (speedup 1.0000x reference)
"""Trainium2 Bass kernel for nn_CausalSelfAttention_22127671509246.

Full (unsharded) inputs in, full output out. Internally shards across 8
NeuronCores: core c handles batch b = c // 4 and head group g = c % 4
(heads 4g..4g+3, i.e. a 256-wide slice of the QKV output channels).

Per-core compute (all matmuls bf16, f32 PSUM accumulation):
  - Q^T, K^T projections in channel-major layout [256, 2048]
  - V projection in row-major layout with a ones column appended per head
    (so the PV matmul also produces the softmax denominator)
  - attention processed in head PAIRS (partition bases 0 and 64) so the
    K=64 QK matmuls overlap in distinct PE row groups
  - attT[k, q] = K^T_h.T @ Q^T_h -> exp(attT / 8) -> SBUF-resident bf16 ax
    buffer -> PV accumulated per 128-row q tile over all k tiles ->
    normalize by the ones-column denominator.

Schedule: the kernel is paced by the PE; ScalarE alone cannot keep up
with the 16.8M-element exp stream (~130us), so exp tiles are split 9/7
between ScalarE (activation Exp) and the DVE (Schraudolph bit-trick:
int16 = logit * 16/ln2 + (127*128 - 7.5), whose bit pattern IS bf16 exp;
1.8% rms on the DVE slice).  Projections are not a separate phase: K/Q
ct0 are pipelined against chunked input DMA at the start so attention
begins at ~6us; all remaining projection matmuls are stuffed into the
attention blocks' PE stream (V psums use the otherwise-idle y banks so
they never steal a QK PSUM-ring slot; ring-stealing Q/K groups are
spaced >=4 kp steps apart).  Dummy matmuls on a zeroed scratch tile
bridge DMA-gated holes in the prologue so the HAM clock gate reaches
8/8 (2.4 GHz) by ~11us and never re-throttles mid-kernel.  PV clumps of
the previous block run at kp steps 2/4/6/post so its trailing exps have
slack.  Constraints learned on HW: PSUM accumulation groups must stay
contiguous per group (interleaving corrupts); the paired 64-row QK
matmuls share rhs-stream bandwidth (~385ns/pair — 4-way K=32 row tiling
is NOT faster since the stream cap, not the array, binds).
Softmax max-subtraction is skipped: logits are ~N(0,1) (max |logit| ~ 7),
so exp never overflows in f32 and softmax is shift-invariant.

Session-2 findings baked in: (1) inputs are HOST-PERMUTED so every DMA is
identity-layout with 2-16KB contiguous lines, and the xT stream is
sc-major in kd-pair chunks matching consumption order (the old per-kd
rearranged transfers had 0.5-1KB lines); (2) output y is bf16 (cast to
f32 on host; +~0.02% rel err) halving output DMA; (3) the final drain
uses 2 rotating 1-bank PSUM tiles with an epilogue per (hh, j-pair)
group, so each 32KB output chunk leaves right as its accumulation group
completes — a per-16-matmul epilogue stalls the PE ~700ns/group on the
same-tile WAR, and giving each (hh,j) its own sub-bank region of ONE
tile corrupts/hangs: two CONCURRENT accumulation groups must not share
a PSUM bank (this also rules out 256-wide QK half-blocks, whose hh1
output would sit mid-bank; N=256 QK pairs also expose the ~95ns
ldweights that N=512 streams hide).  Ideas measured and rejected:
fp8/DoubleRow matmuls (softmax output is a weighted average, so e4m3's
~4% element error passes straight into rel err: 3-8e-2 vs 2e-2 budget);
multi-engine DMA enqueue spreading (early window is bandwidth-bound at
half clock until the HAM ramps ~11.6us — concurrent queues steal
bandwidth from the critical wk/sc0 prefix, +7us).  Fixed NEFF overhead
(preamble ~7.2us + teardown) is ~12-15us of the total; exec floor for
this structure is ~140us.  Runs on a hot device can throttle ~20%
(176us) and even fault spuriously — cool down and re-measure before
trusting a regression.
"""

import os
import sys
import types

sys.path.insert(0, "/opt/trn_rl_repo")

import numpy as np
import ml_dtypes

import concourse.bass as bass
import concourse.bacc as bacc
import concourse.mybir as mybir
import concourse.tile as tile
from concourse.bass import ts

B, S, D = 2, 2048, 1024
H, HD = 16, 64
N_CORES = 8
C = 256           # output channels per core (4 heads)
CT = C // 128     # channel tiles per core
KD = D // 128     # contraction chunks for the projections
SC = S // 512     # 512-wide column chunks of S
STL = S // 128    # 128-row tiles of S
HPC = 4           # heads per core
SCALE = 1.0 / np.sqrt(HD)

# Schraudolph exp on DVE: i16 = raw_logit * (SCALE*128/ln2) + (127*128 - C0)
SCH_C0 = 7.5
SCH_A = float(SCALE * 128.0 / np.log(2.0))
SCH_B = float(127.0 * 128.0 - SCH_C0)
# kt tiles handled by the DVE (rest on ScalarE); spread so each kp step
# keeps both engines fed.  Measured per-tile cost: ScalarE Exp 857ns,
# DVE Schraudolph 1190ns (the PSUM f32 input blocks the DVE 2x mode, and
# op count does not matter).
DVE_KT = frozenset((1, 3, 5, 8, 10, 12, 14))
DVE_KT_LATE = frozenset((1, 3, 5, 7, 9, 11, 13, 15))

F32 = mybir.dt.float32
BF16 = mybir.dt.bfloat16
I16 = mybir.dt.int16

HALF_LAST = False  # split the last qc into two 256-wide half-blocks

_compiled = {}


def _install_ntff_hook():
    """Optional: register the axon NTFF profiling hook if the image lacks it."""
    if "antenv.axon_hooks" in sys.modules:
        return
    try:
        import trn_agent_boot.trn_boot as tb

        mod = types.ModuleType("antenv.axon_hooks")
        hook = tb._ntff_profile_via_ctypes("/opt/axon/libaxon_pjrt.so")
        mod.get_axon_ntff_profile_hook = lambda: hook
        mod.set_axon_ntff_profile_hook = lambda h: None
        sys.modules["antenv.axon_hooks"] = mod
    except Exception:
        pass


def _emit(tc, ctx):
    nc = tc.nc
    # Host-permuted layouts: identity DMAs with multi-KB contiguous lines.
    # xT[p, sc, kd, c] = x.T[kd*128+p, sc*512+c] (16KB lines per (p, sc));
    # w[p, kd, c] = W[kd*128+p, c] (4KB lines per p).
    xT = nc.dram_tensor("xT", [128, SC, KD, 512], BF16,
                        kind="ExternalInput").ap()
    wq = nc.dram_tensor("wq", [128, KD, C], BF16, kind="ExternalInput").ap()
    wk = nc.dram_tensor("wk", [128, KD, C], BF16, kind="ExternalInput").ap()
    wv = nc.dram_tensor("wv", [128, KD, C], BF16, kind="ExternalInput").ap()
    bq = nc.dram_tensor("bq", [C], F32, kind="ExternalInput").ap()
    bk = nc.dram_tensor("bk", [C], F32, kind="ExternalInput").ap()
    bv = nc.dram_tensor("bv", [C], F32, kind="ExternalInput").ap()
    y = nc.dram_tensor("y", [S, C], BF16, kind="ExternalOutput").ap()

    singles = ctx.enter_context(tc.tile_pool(name="singles", bufs=1))
    ax_pool = ctx.enter_context(tc.tile_pool(name="ax", bufs=3))
    yout_pool = ctx.enter_context(tc.tile_pool(name="yout", bufs=3))
    recip_pool = ctx.enter_context(tc.tile_pool(name="recip", bufs=4))
    ps_pool = ctx.enter_context(tc.tile_pool(name="ps", bufs=3, space="PSUM"))
    psy_pool = ctx.enter_context(tc.tile_pool(name="psy", bufs=1, space="PSUM"))

    # ---- SBUF tiles ----
    # xT_sb[p, sc, kd, c]: sc-major so DMA chunks are contiguous and arrive
    # in consumption order (sc0 first, then sc1..sc3).
    xT_sb = singles.tile([128, SC, KD, 512], BF16)
    w_sbs = {}
    w_sbs["k"] = singles.tile([128, KD, C], BF16, tag="wk", name="wk_sb")
    w_sbs["q"] = singles.tile([128, KD, C], BF16, tag="wq", name="wq_sb")
    w_sbs["v"] = singles.tile([128, KD, C], BF16, tag="wv", name="wv_sb")
    bq_sb = singles.tile([128, CT], F32, tag="bq")
    bk_sb = singles.tile([128, CT], F32, tag="bk")

    # DMA order = arrival order on the sync queue.  All transfers are
    # identity-layout (host pre-permuted): weights 4KB lines, xT 2-16KB
    # lines.  sc0 goes in kd-pair chunks so the prologue K projection can
    # start after ~0.7us of x data; sc1..3 as whole-sc transfers.
    warm_sb = singles.tile([128, 512], BF16, tag="warm")
    nc.vector.memset(warm_sb[:], 0.0)

    # Enqueues cost ~650ns each on the issuing engine's sequencer, so they
    # are spread across engines (sync / vector / scalar / gpsimd) to avoid
    # serializing the input stream behind a single queue; order per engine
    # follows consumer need order.
    # Single sync-queue stream in strict need order: the early window is
    # bandwidth-bound (half clock until the HAM ramps ~11.6us), so any
    # concurrent queue steals bandwidth from the critical wk/sc0 prefix.
    nc.sync.dma_start(w_sbs["k"][:], wk)
    nc.sync.dma_start(bk_sb[:], bk.rearrange("(o p) -> p o", p=128))
    for k2 in range(0, KD, 2):
        nc.sync.dma_start(xT_sb[:, 0, k2 : k2 + 2, :], xT[:, 0, k2 : k2 + 2, :])
        if k2 == 0:
            nc.sync.dma_start(w_sbs["q"][:], wq)
            nc.sync.dma_start(bq_sb[:], bq.rearrange("(o p) -> p o", p=128))
    for sc in (1, 2, 3):
        for k2 in range(0, KD, 2):
            nc.sync.dma_start(xT_sb[:, sc, k2 : k2 + 2, :],
                             xT[:, sc, k2 : k2 + 2, :])
        if sc == 1:
            nc.sync.dma_start(w_sbs["v"][:], wv)
    # bv broadcast across partitions (DMA with partition step 0)
    bv_bc = singles.tile([128, C], F32, tag="bvbc")
    bv_bcast_ap = bass.AP(tensor=bv.tensor, offset=bv.offset,
                          ap=[[0, 128]] + list(bv.ap))
    nc.gpsimd.dma_start(out=bv_bc[:], in_=bv_bcast_ap)

    # V with a ones column appended per head: [128, s_tile, head, 65]
    v_sb = singles.tile([128, STL, HPC, HD + 1], BF16, tag="vones")
    nc.vector.memset(v_sb[:, :, :, HD], 1.0)

    qt_sb = singles.tile([128, CT, S], BF16, tag="qt")
    kt_sb = singles.tile([128, CT, S], BF16, tag="kt")

    # HAM warmup: junk matmuls on a zeroed scratch tile into the (not yet
    # used) y0 PSUM bank.  They have no DMA deps, so they keep the PE busy
    # while the prologue projections wait on input DMA — otherwise the PE
    # idles in ~3us chunks and the clock gate holds it at 1.2 GHz for the
    # first ~50us.
    warm_ps = psy_pool.tile([128, 4, HD + 1], F32, tag="y0", name="warm_ps")

    def dummies(n):
        for _ in range(n):
            nc.tensor.matmul(
                warm_ps[:], lhsT=warm_sb[:, 0:128], rhs=warm_sb[:, 0:260],
                start=True, stop=True,
            )

    # ---- projection groups (8 matmuls + 1 bias op each) ----
    def proj_qk(which, ct, sc):
        w_sb = w_sbs[which]
        dst = qt_sb if which == "q" else kt_sb
        bias = bq_sb if which == "q" else bk_sb
        ps = ps_pool.tile([128, 1024], F32, tag="qk", name="ps_proj")
        for kd in range(KD):
            nc.tensor.matmul(
                ps[:, 0:512],
                lhsT=w_sb[:, kd, ts(ct, 128)],
                rhs=xT_sb[:, sc, kd, :],
                start=(kd == 0),
                stop=(kd == KD - 1),
            )
        nc.vector.tensor_scalar_add(
            dst[:, ct, ts(sc, 512)], ps[:, 0:512], bias[:, ct : ct + 1]
        )

    def proj_v(st):
        # V projections run only in block 0, when the y PSUM banks are
        # still idle — use them instead of stealing a QK-ring slot (the
        # ring slot would be held until the bias-add clears the DVE FIFO,
        # stalling QK allocation and starving the exp engines).
        ps = psy_pool.tile([128, 4, HD + 1], F32, tag=f"y{st % 2}",
                           name="vps")
        flat = ps[:].rearrange("p a b -> p (a b)")
        for kd in range(KD):
            nc.tensor.matmul(
                flat[:, 0:C],
                lhsT=xT_sb[:, st // 4, kd, ts(st % 4, 128)],
                rhs=w_sbs["v"][:, kd, :],
                start=(kd == 0),
                stop=(kd == KD - 1),
            )
        nc.vector.tensor_tensor(
            v_sb[:, st, :, 0:HD],
            flat[:, 0:C].rearrange("p (h d) -> p h d", h=HPC),
            bv_bc.rearrange("p (h d) -> p h d", h=HPC),
            mybir.AluOpType.add,
        )

    # ---- attention ----
    # 7 full-width blocks + the last qc slot split into two 256-wide
    # half-blocks, so the final drain (which cannot overlap anything) is
    # half as long.
    if HALF_LAST:
        blocks = [(pair, qc, 0, 512) for pair in range(HPC // 2)
                  for qc in range(SC)][:-1]
        blocks += [(1, 3, 0, 256), (1, 3, 256, 256)]
    else:
        blocks = [(pair, qc, 0, 512) for pair in range(HPC // 2)
                  for qc in range(SC)]

    def qk_exp_block(pair, qc, qoff, w, ax_tile, late=False):
        """Per kp step: 4 QK matmuls (head pair in distinct PE row groups),
        then 2 exps routed to ScalarE or DVE."""
        ct = pair
        q0 = qc * 512 + qoff
        for kp in range(STL // 2):
            tiles = []
            for u in range(2):
                ps = ps_pool.tile([128, 1024], F32, tag="qk", name="ps_att")
                tiles.append(ps)
            for u in range(2):
                kt = 2 * kp + u
                for hh in range(2):
                    p0 = hh * 64
                    # hh1 stays at the 2KB bank boundary: two concurrent
                    # accumulation groups must not share a PSUM bank
                    nc.tensor.matmul(
                        tiles[u][:, hh * 512 : hh * 512 + w],
                        lhsT=kt_sb[p0 : p0 + 64, ct, ts(kt, 128)],
                        rhs=qt_sb[p0 : p0 + 64, ct, q0 : q0 + w],
                        start=True,
                        stop=True,
                    )
            for u in range(2):
                kt = 2 * kp + u
                if w == 512:
                    ax_dst = ax_tile[:, kt, :]
                    ps_src = tiles[u][:]
                else:
                    # gather the two 256-wide head halves (at 0 and 2KB)
                    # into the contiguous 512-wide ax slice
                    ax_dst = ax_tile[:, kt, 0 : 2 * w].rearrange(
                        "p (h c) -> p h c", h=2)
                    ps_src = tiles[u][:].rearrange(
                        "p (h c) -> p h c", h=2)[:, :, 0:w]
                if kt in (DVE_KT_LATE if late else DVE_KT):
                    nc.vector.tensor_scalar(
                        ax_dst.bitcast(I16), ps_src,
                        SCH_A, SCH_B,
                        mybir.AluOpType.mult, mybir.AluOpType.add,
                    )
                else:
                    nc.scalar.activation(
                        ax_dst, ps_src,
                        mybir.ActivationFunctionType.Exp, scale=SCALE,
                    )
            yield

    def pv_mms(pair, w, ax_tile, y_ps):
        """PV matmul closures (hh/j-major, kt innermost: PSUM accumulation
        groups must stay contiguous — interleaving them corrupts)."""
        mms = []
        for hh in range(2):
            h = 2 * pair + hh
            for j in range(w // 128):
                for kt in range(STL):
                    def mm(hh=hh, h=h, j=j, kt=kt):
                        nc.tensor.matmul(
                            y_ps[hh][:, j, :],
                            lhsT=ax_tile[:, kt,
                                         hh * w + j * 128
                                         : hh * w + (j + 1) * 128],
                            rhs=v_sb[:, kt, h, :],
                            start=(kt == 0),
                            stop=(kt == STL - 1),
                        )
                    mms.append(mm)
        return mms

    def epilogue_h(pair, qc, qoff, wj, y_ps, hh):
        # one head's normalize + writeback; called per half so hh0's
        # epilogue overlaps hh1's PV matmuls (and frees its y bank early)
        h = 2 * pair + hh
        q0 = qc * 512 + qoff
        yo = yout_pool.tile([128, wj, HD], BF16, tag="yo", name="yo")
        rc = recip_pool.tile([128, wj], F32, tag="rc", name="rc")
        nc.vector.reciprocal(rc[:], y_ps[hh][:, 0:wj, HD])
        nc.vector.tensor_tensor(
            yo[:],
            y_ps[hh][:, 0:wj, 0:HD],
            rc[:, :, None].to_broadcast((128, wj, HD)),
            mybir.AluOpType.mult,
        )
        nc.sync.dma_start(
            y[q0 : q0 + wj * 128, ts(h, HD)].rearrange(
                "(j p) d -> p j d", p=128
            ),
            yo[:],
        )

    def epilogue_dr(pair, qc, dr, hh, jp):
        # drain-only: normalize + write back a 256-row j-pair slice as
        # soon as its PSUM accumulation group completes, so the final
        # output DMA is a 32KB chunk issued right after the last matmul.
        h = 2 * pair + hh
        yo = yout_pool.tile([128, 2, HD], BF16, tag="yoj", name="yoj")
        rc = recip_pool.tile([128, 2], F32, tag="rcj", name="rcj")
        nc.vector.reciprocal(rc[:], dr[:, :, HD])
        nc.vector.tensor_tensor(
            yo[:],
            dr[:, :, 0:HD],
            rc[:, :, None].to_broadcast((128, 2, HD)),
            mybir.AluOpType.mult,
        )
        nc.sync.dma_start(
            y[ts(2 * qc + jp, 256), ts(h, HD)].rearrange(
                "(j p) d -> p j d", p=128
            ),
            yo[:],
        )

    # ---- schedule ----
    # Prologue: K/Q ct0 sc0 so block (0,0) can start immediately; dummy
    # matmuls ahead of each DMA-gated projection matmul keep the PE warm.
    dummies(12)

    def proj_qk_warm(which, ct, sc, per_kd):
        w_sb = w_sbs[which]
        dst = qt_sb if which == "q" else kt_sb
        bias = bq_sb if which == "q" else bk_sb
        ps = ps_pool.tile([128, 1024], F32, tag="qk", name="ps_proj")
        for kd in range(KD):
            dummies(per_kd)
            nc.tensor.matmul(
                ps[:, 0:512],
                lhsT=w_sb[:, kd, ts(ct, 128)],
                rhs=xT_sb[:, sc, kd, :],
                start=(kd == 0),
                stop=(kd == KD - 1),
            )
        nc.vector.tensor_scalar_add(
            dst[:, ct, ts(sc, 512)], ps[:, 0:512], bias[:, ct : ct + 1]
        )

    proj_qk_warm("k", 0, 0, 4)
    proj_qk_warm("q", 0, 0, 3)

    # Late projection groups stuffed into the attention blocks' PE stream.
    # Need-by: qt[ct0,sc_j] before block j; V s-tiles 0..3 before block 1
    # step 0, 4..7 before step 2, ... (pv is kt-major, clumps at steps
    # 0/2/4/6); kt[ct1]+qt[ct1,sc0] before block 4; qt[ct1,sc_j] before
    # block 4+j.
    stuff = {}
    # block 0: remaining K ct0 groups ride ahead of their QK consumers
    # (kp step s consumes kt tiles 2s,2s+1 -> sc group (2s+1)//4).
    # V groups use the psy banks, so only proj_qk calls steal a QK-ring
    # slot; keep those <=2 per block and >=4 steps apart.
    # K sc1 at step 1, not 0: its first matmul waits on the sc1 k-pair-0
    # DMA (~14.1us), while step 0's QK pairs have all their data — run
    # the ready work first so the PE never idles long enough for the HAM
    # to re-throttle the clock (a ~1.5us gap here costs an extra ~3.4us
    # of half-clock).
    stuff[(0, 1)] = [lambda: proj_qk("k", 0, 1)]
    stuff[(0, 2)] = [lambda: proj_qk("k", 0, 2)]
    stuff[(0, 3)] = [lambda: proj_v(0), lambda: proj_v(1),
                     lambda: proj_v(2)]
    stuff[(0, 4)] = [lambda: proj_qk("k", 0, 3),
                     lambda: proj_v(3), lambda: proj_v(4),
                     lambda: proj_v(5)]
    stuff[(0, 5)] = [lambda: proj_v(6), lambda: proj_v(7),
                     lambda: proj_v(8)]
    stuff[(0, 6)] = [lambda: proj_v(9), lambda: proj_v(10),
                     lambda: proj_v(11), lambda: proj_v(12)]
    stuff[(0, 7)] = [lambda: proj_qk("q", 0, 1),
                     lambda: proj_v(13), lambda: proj_v(14),
                     lambda: proj_v(15)]
    stuff[(1, 2)] = [lambda: proj_qk("q", 0, 2)]
    stuff[(1, 6)] = [lambda: proj_qk("k", 1, 0)]
    stuff[(2, 1)] = [lambda: proj_qk("q", 0, 3)]
    stuff[(2, 5)] = [lambda: proj_qk("k", 1, 1)]
    stuff[(3, 1)] = [lambda: proj_qk("k", 1, 2)]
    stuff[(3, 5)] = [lambda: proj_qk("q", 1, 0)]
    stuff[(4, 1)] = [lambda: proj_qk("k", 1, 3)]
    stuff[(4, 5)] = [lambda: proj_qk("q", 1, 1)]
    stuff[(5, 1)] = [lambda: proj_qk("q", 1, 2)]
    stuff[(6, 1)] = [lambda: proj_qk("q", 1, 3)]

    def alloc_y(wj):
        return [psy_pool.tile([128, wj, HD + 1], F32, tag=f"y{hh}",
                              name=f"y{hh}") for hh in range(2)]

    prev = None  # (pair, qc, qoff, w, ax_tile)
    prev_y = None
    for i, (pair, qc, qoff, w) in enumerate(blocks):
        ax_tile = ax_pool.tile([128, STL, 1024], BF16, tag="ax", name="ax")
        if prev:
            prev_y = alloc_y(prev[3] // 128)
            pv_prev = pv_mms(prev[0], prev[3], prev[4], prev_y)
        else:
            pv_prev = []
        L = len(pv_prev)
        q8 = L // 8
        step = 0
        gen = qk_exp_block(pair, qc, qoff, w, ax_tile,
                           late=(i == len(blocks) - 1))
        while True:
            # block 0 is DMA-paced: dummies BEFORE the stuffed (DMA-gated)
            # projections, front-loaded where the sc1/sc2 waits live
            if i == 0 and step < 8:
                dummies((8, 8, 4, 4, 2, 2, 2, 2)[step])
            for fn in stuff.get((i, step), ()):
                fn()
            # PV matmuls of the previous block, 2*q8 per two kp steps,
            # starting at step 2 so the previous block's trailing exps
            # have slack (the first PV group reads every kt tile).  Keep
            # the clumps COARSE: every QK->PV transition pays ~280ns of
            # pair-stream serialization, so fewer, bigger clumps win over
            # spreading (measured +5us for one-group-per-step).
            if step in (2, 4, 6) and L:
                for mm in pv_prev[q8 * (step - 2) : q8 * step]:
                    mm()
                if step == 4:
                    # hh0's accumulation groups (pv[0:L/2]) are complete
                    epilogue_h(prev[0], prev[1], prev[2], prev[3] // 128,
                               prev_y, 0)
            if next(gen, "done") == "done":
                break
            step += 1
        for mm in pv_prev[q8 * 6 :]:
            mm()
        if prev:
            epilogue_h(prev[0], prev[1], prev[2], prev[3] // 128, prev_y, 1)
        prev = (pair, qc, qoff, w, ax_tile)
    # Drain the last (256-wide) half-block into the y0/y1 banks (free
    # once the previous half-block's epilogue cleared), one accumulation
    # group per head (2 j slices x 16 kt), epilogue per group: the DVE
    # normalize + 32KB DMA of head hh0 overlap head hh1's matmuls.
    pair, qc, qoff, w, ax_tile = prev
    if HALF_LAST:
        for hh in range(2):
            h = 2 * pair + hh
            dr = psy_pool.tile([128, 2, HD + 1], F32, tag=f"y{hh}",
                               name=f"dr{hh}")
            for j in range(2):
                for kt in range(STL):
                    nc.tensor.matmul(
                        dr[:, j, :],
                        lhsT=ax_tile[:, kt,
                                     hh * w + j * 128
                                     : hh * w + (j + 1) * 128],
                        rhs=v_sb[:, kt, h, :],
                        start=(kt == 0),
                        stop=(kt == STL - 1),
                    )
            epilogue_dr(pair, qc, dr, hh, 1)
    else:
        for g in range(4):
            hh, jp = divmod(g, 2)
            h = 2 * pair + hh
            dr = psy_pool.tile([128, 2, HD + 1], F32, tag=f"y{g % 2}",
                               name=f"dr{g}")
            for jj in range(2):
                j = 2 * jp + jj
                for kt in range(STL):
                    nc.tensor.matmul(
                        dr[:, jj, :],
                        lhsT=ax_tile[:, kt,
                                     hh * 512 + j * 128
                                     : hh * 512 + (j + 1) * 128],
                        rhs=v_sb[:, kt, h, :],
                        start=(kt == 0),
                        stop=(kt == STL - 1),
                    )
            epilogue_dr(pair, qc, dr, hh, jp)


def _build():
    if "nc" in _compiled:
        return _compiled["nc"]
    nc = bacc.Bacc("TRN2", target_bir_lowering=False, debug=False,
                   num_devices=N_CORES)
    from contextlib import ExitStack
    with tile.TileContext(nc) as tc, ExitStack() as ctx:
        _emit(tc, ctx)
    nc.compile()
    _compiled["nc"] = nc
    return nc


def kernel(x, Wq, bq, Wk, bk, Wv, bv, _profile=False):
    x = np.asarray(x, dtype=np.float32)
    Wq = np.asarray(Wq, dtype=np.float32)
    Wk = np.asarray(Wk, dtype=np.float32)
    Wv = np.asarray(Wv, dtype=np.float32)
    bq = np.asarray(bq, dtype=np.float32)
    bk = np.asarray(bk, dtype=np.float32)
    bv = np.asarray(bv, dtype=np.float32)

    nc = _build()

    bf = ml_dtypes.bfloat16
    # xTp[p, sc, kd, c] = x[b].T[kd*128+p, sc*512+c]
    xTp = [
        np.ascontiguousarray(
            x[b].T.reshape(KD, 128, SC, 512).transpose(1, 2, 0, 3)
        ).astype(bf)
        for b in range(B)
    ]

    def wperm(W, sl):
        # w[p, kd, c] = W[kd*128+p, sl][c]
        return np.ascontiguousarray(
            W[:, sl].reshape(KD, 128, C).transpose(1, 0, 2)
        ).astype(bf)

    in_maps = []
    for c in range(N_CORES):
        b, g = divmod(c, HPC)
        sl = slice(g * C, (g + 1) * C)
        in_maps.append({
            "xT": xTp[b],
            "wq": wperm(Wq, sl),
            "wk": wperm(Wk, sl),
            "wv": wperm(Wv, sl),
            "bq": np.ascontiguousarray(bq[sl]),
            "bk": np.ascontiguousarray(bk[sl]),
            "bv": np.ascontiguousarray(bv[sl]),
        })

    from concourse.bass_utils import run_bass_kernel_spmd

    if _profile:
        _install_ntff_hook()
    res = run_bass_kernel_spmd(nc, in_maps, list(range(N_CORES)),
                               trace=_profile)
    out = np.empty((B, S, D), dtype=np.float32)
    for c in range(N_CORES):
        b, g = divmod(c, HPC)
        out[b, :, g * C : (g + 1) * C] = res.results[c]["y"].astype(np.float32)
    if _profile:
        kernel.last_exec_time_ns = res.exec_time_ns
    return out



# revision 45
# speedup vs baseline: 1.0104x; 1.0104x over previous
"""Trainium2 Bass kernel for nn_CausalSelfAttention_22127671509246.

Full (unsharded) inputs in, full output out. Internally shards across 8
NeuronCores: core c handles batch b = c // 4 and head group g = c % 4
(heads 4g..4g+3, i.e. a 256-wide slice of the QKV output channels).

Per-core compute (all matmuls bf16, f32 PSUM accumulation):
  - Q^T, K^T projections in channel-major layout [256, 2048]
  - V projection in row-major layout with a ones column appended per head
    (so the PV matmul also produces the softmax denominator)
  - attention processed in head PAIRS (partition bases 0 and 64) so the
    K=64 QK matmuls overlap in distinct PE row groups
  - attT[k, q] = K^T_h.T @ Q^T_h -> exp(attT / 8) -> SBUF-resident bf16 ax
    buffer -> PV accumulated per 128-row q tile over all k tiles ->
    normalize by the ones-column denominator.

Schedule: the kernel is paced by the PE; ScalarE alone cannot keep up
with the 16.8M-element exp stream (~130us), so exp tiles are split 9/7
between ScalarE (activation Exp) and the DVE (Schraudolph bit-trick:
int16 = logit * 16/ln2 + (127*128 - 7.5), whose bit pattern IS bf16 exp;
1.8% rms on the DVE slice).  Projections are not a separate phase: K/Q
ct0 are pipelined against chunked input DMA at the start so attention
begins at ~6us; all remaining projection matmuls are stuffed into the
attention blocks' PE stream (V psums use the otherwise-idle y banks so
they never steal a QK PSUM-ring slot; ring-stealing Q/K groups are
spaced >=4 kp steps apart).  Dummy matmuls on a zeroed scratch tile
bridge DMA-gated holes in the prologue so the HAM clock gate reaches
8/8 (2.4 GHz) by ~11us and never re-throttles mid-kernel.  PV clumps of
the previous block run at kp steps 2/4/6/post so its trailing exps have
slack.  Constraints learned on HW: PSUM accumulation groups must stay
contiguous per group (interleaving corrupts); the paired 64-row QK
matmuls share rhs-stream bandwidth (~385ns/pair — 4-way K=32 row tiling
is NOT faster since the stream cap, not the array, binds).
Softmax max-subtraction is skipped: logits are ~N(0,1) (max |logit| ~ 7),
so exp never overflows in f32 and softmax is shift-invariant.

Session-2 findings baked in: (1) inputs are HOST-PERMUTED so every DMA is
identity-layout with 2-16KB contiguous lines, and the xT stream is
sc-major in kd-pair chunks matching consumption order (the old per-kd
rearranged transfers had 0.5-1KB lines); (2) output y is bf16 (cast to
f32 on host; +~0.02% rel err) halving output DMA; (3) the final drain
uses 2 rotating 1-bank PSUM tiles with an epilogue per (hh, j-pair)
group, so each 32KB output chunk leaves right as its accumulation group
completes — a per-16-matmul epilogue stalls the PE ~700ns/group on the
same-tile WAR, and giving each (hh,j) its own sub-bank region of ONE
tile corrupts/hangs: two CONCURRENT accumulation groups must not share
a PSUM bank (this also rules out 256-wide QK half-blocks, whose hh1
output would sit mid-bank; N=256 QK pairs also expose the ~95ns
ldweights that N=512 streams hide).  Ideas measured and rejected:
fp8/DoubleRow matmuls (softmax output is a weighted average, so e4m3's
~4% element error passes straight into rel err: 3-8e-2 vs 2e-2 budget);
multi-engine DMA enqueue spreading (early window is bandwidth-bound at
half clock until the HAM ramps ~11.6us — concurrent queues steal
bandwidth from the critical wk/sc0 prefix, +7us).  Fixed NEFF overhead
(preamble ~7.2us + teardown) is ~12-15us of the total; exec floor for
this structure is ~140us.  Runs on a hot device can throttle ~20%
(176us) and even fault spuriously — cool down and re-measure before
trusting a regression.

Session-3 (trace-driven, partially throttle-limited measurement):
steady-state QK pairs issue concurrently (~310ns/pair wall) and some
wait on $S[163] (DVE) — the QK PSUM ring (3 tiles) is freed by exp
completions and the DVE Schraudolph tile costs 1190ns (PSUM f32 input
blocks its 2x mode; op count is free, so pre-scaling K by SCH_A buys
nothing).  Spreading PV one-group-per-step to relieve the ring LOSES
~5us: every QK->PV transition pays ~280ns of pair-stream serialization,
so coarse clumps at steps 2/4/6 win.  Block-0 dummies now run BEFORE
the DMA-gated stuffed projections and K-sc1 moved to step 1 so ready QK
work runs first: HAM reaches 8/8 by ~10.9us with no mid-kernel
re-throttle (a ~1.5us prologue gap can trigger a 3.4us half-clock
episode).
"""

import os
import sys
import types

sys.path.insert(0, "/opt/trn_rl_repo")

import numpy as np
import ml_dtypes

import concourse.bass as bass
import concourse.bacc as bacc
import concourse.mybir as mybir
import concourse.tile as tile
from concourse.bass import ts

B, S, D = 2, 2048, 1024
H, HD = 16, 64
N_CORES = 8
C = 256           # output channels per core (4 heads)
CT = C // 128     # channel tiles per core
KD = D // 128     # contraction chunks for the projections
SC = S // 512     # 512-wide column chunks of S
STL = S // 128    # 128-row tiles of S
HPC = 4           # heads per core
SCALE = 1.0 / np.sqrt(HD)

# Schraudolph exp on DVE: i16 = raw_logit * (SCALE*128/ln2) + (127*128 - C0)
SCH_C0 = 7.5
SCH_A = float(SCALE * 128.0 / np.log(2.0))
SCH_B = float(127.0 * 128.0 - SCH_C0)
# kt tiles handled by the DVE (rest on ScalarE); spread so each kp step
# keeps both engines fed.  Measured per-tile cost: ScalarE Exp 857ns,
# DVE Schraudolph 1190ns (the PSUM f32 input blocks the DVE 2x mode, and
# op count does not matter).
DVE_KT = frozenset((1, 3, 5, 8, 10, 12, 14))
DVE_KT_LATE = frozenset((1, 3, 5, 7, 9, 11, 13, 15))

F32 = mybir.dt.float32
BF16 = mybir.dt.bfloat16
I16 = mybir.dt.int16

HALF_LAST = False  # split the last qc into two 256-wide half-blocks

_compiled = {}


def _install_ntff_hook():
    """Optional: register the axon NTFF profiling hook if the image lacks it."""
    if "antenv.axon_hooks" in sys.modules:
        return
    try:
        import trn_agent_boot.trn_boot as tb

        mod = types.ModuleType("antenv.axon_hooks")
        hook = tb._ntff_profile_via_ctypes("/opt/axon/libaxon_pjrt.so")
        mod.get_axon_ntff_profile_hook = lambda: hook
        mod.set_axon_ntff_profile_hook = lambda h: None
        sys.modules["antenv.axon_hooks"] = mod
    except Exception:
        pass


def _emit(tc, ctx):
    nc = tc.nc
    # Host-permuted layouts: identity DMAs with multi-KB contiguous lines.
    # xT[p, sc, kd, c] = x.T[kd*128+p, sc*512+c] (16KB lines per (p, sc));
    # w[p, kd, c] = W[kd*128+p, c] (4KB lines per p).
    xT = nc.dram_tensor("xT", [128, SC, KD, 512], BF16,
                        kind="ExternalInput").ap()
    wq = nc.dram_tensor("wq", [128, KD, C], BF16, kind="ExternalInput").ap()
    wk = nc.dram_tensor("wk", [128, KD, C], BF16, kind="ExternalInput").ap()
    wv = nc.dram_tensor("wv", [128, KD, C], BF16, kind="ExternalInput").ap()
    bq = nc.dram_tensor("bq", [C], F32, kind="ExternalInput").ap()
    bk = nc.dram_tensor("bk", [C], F32, kind="ExternalInput").ap()
    bv = nc.dram_tensor("bv", [C], F32, kind="ExternalInput").ap()
    y = nc.dram_tensor("y", [S, C], BF16, kind="ExternalOutput").ap()

    singles = ctx.enter_context(tc.tile_pool(name="singles", bufs=1))
    ax_pool = ctx.enter_context(tc.tile_pool(name="ax", bufs=3))
    yout_pool = ctx.enter_context(tc.tile_pool(name="yout", bufs=3))
    recip_pool = ctx.enter_context(tc.tile_pool(name="recip", bufs=4))
    ps_pool = ctx.enter_context(tc.tile_pool(name="ps", bufs=3, space="PSUM"))
    psy_pool = ctx.enter_context(tc.tile_pool(name="psy", bufs=1, space="PSUM"))

    # ---- SBUF tiles ----
    # xT_sb[p, sc, kd, c]: sc-major so DMA chunks are contiguous and arrive
    # in consumption order (sc0 first, then sc1..sc3).
    xT_sb = singles.tile([128, SC, KD, 512], BF16)
    w_sbs = {}
    w_sbs["k"] = singles.tile([128, KD, C], BF16, tag="wk", name="wk_sb")
    w_sbs["q"] = singles.tile([128, KD, C], BF16, tag="wq", name="wq_sb")
    w_sbs["v"] = singles.tile([128, KD, C], BF16, tag="wv", name="wv_sb")
    bq_sb = singles.tile([128, CT], F32, tag="bq")
    bk_sb = singles.tile([128, CT], F32, tag="bk")

    # DMA order = arrival order on the sync queue.  All transfers are
    # identity-layout (host pre-permuted): weights 4KB lines, xT 2-16KB
    # lines.  sc0 goes in kd-pair chunks so the prologue K projection can
    # start after ~0.7us of x data; sc1..3 as whole-sc transfers.
    warm_sb = singles.tile([128, 512], BF16, tag="warm")
    nc.vector.memset(warm_sb[:], 0.0)

    # Enqueues cost ~650ns each on the issuing engine's sequencer, so they
    # are spread across engines (sync / vector / scalar / gpsimd) to avoid
    # serializing the input stream behind a single queue; order per engine
    # follows consumer need order.
    # Single sync-queue stream in strict need order: the early window is
    # bandwidth-bound (half clock until the HAM ramps ~11.6us), so any
    # concurrent queue steals bandwidth from the critical wk/sc0 prefix.
    nc.sync.dma_start(w_sbs["k"][:], wk)
    nc.sync.dma_start(bk_sb[:], bk.rearrange("(o p) -> p o", p=128))
    for k2 in range(0, KD, 2):
        nc.sync.dma_start(xT_sb[:, 0, k2 : k2 + 2, :], xT[:, 0, k2 : k2 + 2, :])
        if k2 == 0:
            nc.sync.dma_start(w_sbs["q"][:], wq)
            nc.sync.dma_start(bq_sb[:], bq.rearrange("(o p) -> p o", p=128))
    for sc in (1, 2, 3):
        for k2 in range(0, KD, 2):
            nc.sync.dma_start(xT_sb[:, sc, k2 : k2 + 2, :],
                             xT[:, sc, k2 : k2 + 2, :])
        if sc == 1:
            nc.sync.dma_start(w_sbs["v"][:], wv)
    # bv broadcast across partitions (DMA with partition step 0)
    bv_bc = singles.tile([128, C], F32, tag="bvbc")
    bv_bcast_ap = bass.AP(tensor=bv.tensor, offset=bv.offset,
                          ap=[[0, 128]] + list(bv.ap))
    nc.gpsimd.dma_start(out=bv_bc[:], in_=bv_bcast_ap)

    # V with a ones column appended per head: [128, s_tile, head, 65]
    v_sb = singles.tile([128, STL, HPC, HD + 1], BF16, tag="vones")
    nc.vector.memset(v_sb[:, :, :, HD], 1.0)

    qt_sb = singles.tile([128, CT, S], BF16, tag="qt")
    kt_sb = singles.tile([128, CT, S], BF16, tag="kt")

    # HAM warmup: junk matmuls on a zeroed scratch tile into the (not yet
    # used) y0 PSUM bank.  They have no DMA deps, so they keep the PE busy
    # while the prologue projections wait on input DMA — otherwise the PE
    # idles in ~3us chunks and the clock gate holds it at 1.2 GHz for the
    # first ~50us.
    warm_ps = psy_pool.tile([128, 4, HD + 1], F32, tag="y0", name="warm_ps")

    def dummies(n):
        for _ in range(n):
            nc.tensor.matmul(
                warm_ps[:], lhsT=warm_sb[:, 0:128], rhs=warm_sb[:, 0:260],
                start=True, stop=True,
            )

    # ---- projection groups (8 matmuls + 1 bias op each) ----
    def proj_qk(which, ct, sc):
        w_sb = w_sbs[which]
        dst = qt_sb if which == "q" else kt_sb
        bias = bq_sb if which == "q" else bk_sb
        ps = ps_pool.tile([128, 1024], F32, tag="qk", name="ps_proj")
        for kd in range(KD):
            nc.tensor.matmul(
                ps[:, 0:512],
                lhsT=w_sb[:, kd, ts(ct, 128)],
                rhs=xT_sb[:, sc, kd, :],
                start=(kd == 0),
                stop=(kd == KD - 1),
            )
        nc.vector.tensor_scalar_add(
            dst[:, ct, ts(sc, 512)], ps[:, 0:512], bias[:, ct : ct + 1]
        )

    def proj_v(st):
        # V projections run only in block 0, when the y PSUM banks are
        # still idle — use them instead of stealing a QK-ring slot (the
        # ring slot would be held until the bias-add clears the DVE FIFO,
        # stalling QK allocation and starving the exp engines).
        ps = psy_pool.tile([128, 4, HD + 1], F32, tag=f"y{st % 2}",
                           name="vps")
        flat = ps[:].rearrange("p a b -> p (a b)")
        for kd in range(KD):
            nc.tensor.matmul(
                flat[:, 0:C],
                lhsT=xT_sb[:, st // 4, kd, ts(st % 4, 128)],
                rhs=w_sbs["v"][:, kd, :],
                start=(kd == 0),
                stop=(kd == KD - 1),
            )
        nc.vector.tensor_tensor(
            v_sb[:, st, :, 0:HD],
            flat[:, 0:C].rearrange("p (h d) -> p h d", h=HPC),
            bv_bc.rearrange("p (h d) -> p h d", h=HPC),
            mybir.AluOpType.add,
        )

    # ---- attention ----
    # 7 full-width blocks + the last qc slot split into two 256-wide
    # half-blocks, so the final drain (which cannot overlap anything) is
    # half as long.
    if HALF_LAST:
        blocks = [(pair, qc, 0, 512) for pair in range(HPC // 2)
                  for qc in range(SC)][:-1]
        blocks += [(1, 3, 0, 256), (1, 3, 256, 256)]
    else:
        blocks = [(pair, qc, 0, 512) for pair in range(HPC // 2)
                  for qc in range(SC)]

    def qk_exp_block(pair, qc, qoff, w, ax_tile, late=False):
        """Per kp step: 4 QK matmuls (head pair in distinct PE row groups),
        then 2 exps routed to ScalarE or DVE."""
        ct = pair
        q0 = qc * 512 + qoff
        for kp in range(STL // 2):
            tiles = []
            for u in range(2):
                ps = ps_pool.tile([128, 1024], F32, tag="qk", name="ps_att")
                tiles.append(ps)
            for u in range(2):
                kt = 2 * kp + u
                for hh in range(2):
                    p0 = hh * 64
                    # hh1 stays at the 2KB bank boundary: two concurrent
                    # accumulation groups must not share a PSUM bank
                    nc.tensor.matmul(
                        tiles[u][:, hh * 512 : hh * 512 + w],
                        lhsT=kt_sb[p0 : p0 + 64, ct, ts(kt, 128)],
                        rhs=qt_sb[p0 : p0 + 64, ct, q0 : q0 + w],
                        start=True,
                        stop=True,
                    )
            for u in range(2):
                kt = 2 * kp + u
                if w == 512:
                    ax_dst = ax_tile[:, kt, :]
                    ps_src = tiles[u][:]
                else:
                    # gather the two 256-wide head halves (at 0 and 2KB)
                    # into the contiguous 512-wide ax slice
                    ax_dst = ax_tile[:, kt, 0 : 2 * w].rearrange(
                        "p (h c) -> p h c", h=2)
                    ps_src = tiles[u][:].rearrange(
                        "p (h c) -> p h c", h=2)[:, :, 0:w]
                if kt in (DVE_KT_LATE if late else DVE_KT):
                    nc.vector.tensor_scalar(
                        ax_dst.bitcast(I16), ps_src,
                        SCH_A, SCH_B,
                        mybir.AluOpType.mult, mybir.AluOpType.add,
                    )
                else:
                    nc.scalar.activation(
                        ax_dst, ps_src,
                        mybir.ActivationFunctionType.Exp, scale=SCALE,
                    )
            yield

    def pv_mms(pair, w, ax_tile, y_ps):
        """PV matmul closures (hh/j-major, kt innermost: PSUM accumulation
        groups must stay contiguous — interleaving them corrupts)."""
        mms = []
        for hh in range(2):
            h = 2 * pair + hh
            for j in range(w // 128):
                for kt in range(STL):
                    def mm(hh=hh, h=h, j=j, kt=kt):
                        nc.tensor.matmul(
                            y_ps[hh][:, j, :],
                            lhsT=ax_tile[:, kt,
                                         hh * w + j * 128
                                         : hh * w + (j + 1) * 128],
                            rhs=v_sb[:, kt, h, :],
                            start=(kt == 0),
                            stop=(kt == STL - 1),
                        )
                    mms.append(mm)
        return mms

    def epilogue_h(pair, qc, qoff, wj, y_ps, hh):
        # one head's normalize + writeback; called per half so hh0's
        # epilogue overlaps hh1's PV matmuls (and frees its y bank early)
        h = 2 * pair + hh
        q0 = qc * 512 + qoff
        yo = yout_pool.tile([128, wj, HD], BF16, tag="yo", name="yo")
        rc = recip_pool.tile([128, wj], F32, tag="rc", name="rc")
        nc.vector.reciprocal(rc[:], y_ps[hh][:, 0:wj, HD])
        nc.vector.tensor_tensor(
            yo[:],
            y_ps[hh][:, 0:wj, 0:HD],
            rc[:, :, None].to_broadcast((128, wj, HD)),
            mybir.AluOpType.mult,
        )
        nc.sync.dma_start(
            y[q0 : q0 + wj * 128, ts(h, HD)].rearrange(
                "(j p) d -> p j d", p=128
            ),
            yo[:],
        )

    def epilogue_dr(pair, qc, dr, hh, jp):
        # drain-only: normalize + write back a 256-row j-pair slice as
        # soon as its PSUM accumulation group completes, so the final
        # output DMA is a 32KB chunk issued right after the last matmul.
        h = 2 * pair + hh
        yo = yout_pool.tile([128, 2, HD], BF16, tag="yoj", name="yoj")
        rc = recip_pool.tile([128, 2], F32, tag="rcj", name="rcj")
        nc.vector.reciprocal(rc[:], dr[:, :, HD])
        nc.vector.tensor_tensor(
            yo[:],
            dr[:, :, 0:HD],
            rc[:, :, None].to_broadcast((128, 2, HD)),
            mybir.AluOpType.mult,
        )
        nc.sync.dma_start(
            y[ts(2 * qc + jp, 256), ts(h, HD)].rearrange(
                "(j p) d -> p j d", p=128
            ),
            yo[:],
        )

    # ---- schedule ----
    # Prologue: K/Q ct0 sc0 so block (0,0) can start immediately; dummy
    # matmuls ahead of each DMA-gated projection matmul keep the PE warm.
    dummies(12)

    def proj_qk_warm(which, ct, sc, per_kd):
        w_sb = w_sbs[which]
        dst = qt_sb if which == "q" else kt_sb
        bias = bq_sb if which == "q" else bk_sb
        ps = ps_pool.tile([128, 1024], F32, tag="qk", name="ps_proj")
        for kd in range(KD):
            dummies(per_kd)
            nc.tensor.matmul(
                ps[:, 0:512],
                lhsT=w_sb[:, kd, ts(ct, 128)],
                rhs=xT_sb[:, sc, kd, :],
                start=(kd == 0),
                stop=(kd == KD - 1),
            )
        nc.vector.tensor_scalar_add(
            dst[:, ct, ts(sc, 512)], ps[:, 0:512], bias[:, ct : ct + 1]
        )

    proj_qk_warm("k", 0, 0, 3)
    proj_qk_warm("q", 0, 0, 1)

    # Late projection groups stuffed into the attention blocks' PE stream.
    # Need-by: qt[ct0,sc_j] before block j; V s-tiles 0..3 before block 1
    # step 0, 4..7 before step 2, ... (pv is kt-major, clumps at steps
    # 0/2/4/6); kt[ct1]+qt[ct1,sc0] before block 4; qt[ct1,sc_j] before
    # block 4+j.
    stuff = {}
    # block 0: remaining K ct0 groups ride ahead of their QK consumers
    # (kp step s consumes kt tiles 2s,2s+1 -> sc group (2s+1)//4).
    # V groups use the psy banks, so only proj_qk calls steal a QK-ring
    # slot; keep those <=2 per block and >=4 steps apart.
    # K sc1 at step 1, not 0: its first matmul waits on the sc1 k-pair-0
    # DMA (~14.1us), while step 0's QK pairs have all their data — run
    # the ready work first so the PE never idles long enough for the HAM
    # to re-throttle the clock (a ~1.5us gap here costs an extra ~3.4us
    # of half-clock).
    stuff[(0, 1)] = [lambda: proj_qk("k", 0, 1)]
    stuff[(0, 2)] = [lambda: proj_qk("k", 0, 2)]
    stuff[(0, 3)] = [lambda: proj_v(0), lambda: proj_v(1),
                     lambda: proj_v(2)]
    stuff[(0, 4)] = [lambda: proj_qk("k", 0, 3),
                     lambda: proj_v(3), lambda: proj_v(4),
                     lambda: proj_v(5)]
    stuff[(0, 5)] = [lambda: proj_v(6), lambda: proj_v(7),
                     lambda: proj_v(8)]
    stuff[(0, 6)] = [lambda: proj_v(9), lambda: proj_v(10),
                     lambda: proj_v(11), lambda: proj_v(12)]
    stuff[(0, 7)] = [lambda: proj_qk("q", 0, 1),
                     lambda: proj_v(13), lambda: proj_v(14),
                     lambda: proj_v(15)]
    stuff[(1, 2)] = [lambda: proj_qk("q", 0, 2)]
    stuff[(1, 6)] = [lambda: proj_qk("k", 1, 0)]
    stuff[(2, 1)] = [lambda: proj_qk("q", 0, 3)]
    stuff[(2, 5)] = [lambda: proj_qk("k", 1, 1)]
    stuff[(3, 1)] = [lambda: proj_qk("k", 1, 2)]
    stuff[(3, 5)] = [lambda: proj_qk("q", 1, 0)]
    stuff[(4, 1)] = [lambda: proj_qk("k", 1, 3)]
    stuff[(4, 5)] = [lambda: proj_qk("q", 1, 1)]
    stuff[(5, 1)] = [lambda: proj_qk("q", 1, 2)]
    stuff[(6, 1)] = [lambda: proj_qk("q", 1, 3)]

    def alloc_y(wj):
        return [psy_pool.tile([128, wj, HD + 1], F32, tag=f"y{hh}",
                              name=f"y{hh}") for hh in range(2)]

    prev = None  # (pair, qc, qoff, w, ax_tile)
    prev_y = None
    for i, (pair, qc, qoff, w) in enumerate(blocks):
        ax_tile = ax_pool.tile([128, STL, 1024], BF16, tag="ax", name="ax")
        if prev:
            prev_y = alloc_y(prev[3] // 128)
            pv_prev = pv_mms(prev[0], prev[3], prev[4], prev_y)
        else:
            pv_prev = []
        L = len(pv_prev)
        q8 = L // 8
        step = 0
        gen = qk_exp_block(pair, qc, qoff, w, ax_tile,
                           late=(i == len(blocks) - 1))
        while True:
            # block 0 is DMA-paced: dummies BEFORE the stuffed (DMA-gated)
            # projections, front-loaded where the sc1/sc2 waits live
            if i == 0 and step < 8:
                dummies((8, 8, 4, 4, 2, 2, 2, 2)[step])
            for fn in stuff.get((i, step), ()):
                fn()
            # PV matmuls of the previous block, 2*q8 per two kp steps,
            # starting at step 2 so the previous block's trailing exps
            # have slack (the first PV group reads every kt tile).  Keep
            # the clumps COARSE: every QK->PV transition pays ~280ns of
            # pair-stream serialization, so fewer, bigger clumps win over
            # spreading (measured +5us for one-group-per-step).
            if step in (2, 4, 6) and L:
                for mm in pv_prev[q8 * (step - 2) : q8 * step]:
                    mm()
                if step == 4:
                    # hh0's accumulation groups (pv[0:L/2]) are complete
                    epilogue_h(prev[0], prev[1], prev[2], prev[3] // 128,
                               prev_y, 0)
            if next(gen, "done") == "done":
                break
            step += 1
        for mm in pv_prev[q8 * 6 :]:
            mm()
        if prev:
            epilogue_h(prev[0], prev[1], prev[2], prev[3] // 128, prev_y, 1)
        prev = (pair, qc, qoff, w, ax_tile)
    # Drain the last (256-wide) half-block into the y0/y1 banks (free
    # once the previous half-block's epilogue cleared), one accumulation
    # group per head (2 j slices x 16 kt), epilogue per group: the DVE
    # normalize + 32KB DMA of head hh0 overlap head hh1's matmuls.
    pair, qc, qoff, w, ax_tile = prev
    if HALF_LAST:
        for hh in range(2):
            h = 2 * pair + hh
            dr = psy_pool.tile([128, 2, HD + 1], F32, tag=f"y{hh}",
                               name=f"dr{hh}")
            for j in range(2):
                for kt in range(STL):
                    nc.tensor.matmul(
                        dr[:, j, :],
                        lhsT=ax_tile[:, kt,
                                     hh * w + j * 128
                                     : hh * w + (j + 1) * 128],
                        rhs=v_sb[:, kt, h, :],
                        start=(kt == 0),
                        stop=(kt == STL - 1),
                    )
            epilogue_dr(pair, qc, dr, hh, 1)
    else:
        for g in range(4):
            hh, jp = divmod(g, 2)
            h = 2 * pair + hh
            dr = psy_pool.tile([128, 2, HD + 1], F32, tag=f"y{g % 2}",
                               name=f"dr{g}")
            for jj in range(2):
                j = 2 * jp + jj
                for kt in range(STL):
                    nc.tensor.matmul(
                        dr[:, jj, :],
                        lhsT=ax_tile[:, kt,
                                     hh * 512 + j * 128
                                     : hh * 512 + (j + 1) * 128],
                        rhs=v_sb[:, kt, h, :],
                        start=(kt == 0),
                        stop=(kt == STL - 1),
                    )
            epilogue_dr(pair, qc, dr, hh, jp)


def _build():
    if "nc" in _compiled:
        return _compiled["nc"]
    nc = bacc.Bacc("TRN2", target_bir_lowering=False, debug=False,
                   num_devices=N_CORES)
    from contextlib import ExitStack
    with tile.TileContext(nc) as tc, ExitStack() as ctx:
        _emit(tc, ctx)
    nc.compile()
    _compiled["nc"] = nc
    return nc


def kernel(x, Wq, bq, Wk, bk, Wv, bv, _profile=False):
    x = np.asarray(x, dtype=np.float32)
    Wq = np.asarray(Wq, dtype=np.float32)
    Wk = np.asarray(Wk, dtype=np.float32)
    Wv = np.asarray(Wv, dtype=np.float32)
    bq = np.asarray(bq, dtype=np.float32)
    bk = np.asarray(bk, dtype=np.float32)
    bv = np.asarray(bv, dtype=np.float32)

    nc = _build()

    bf = ml_dtypes.bfloat16
    # xTp[p, sc, kd, c] = x[b].T[kd*128+p, sc*512+c]
    xTp = [
        np.ascontiguousarray(
            x[b].T.reshape(KD, 128, SC, 512).transpose(1, 2, 0, 3)
        ).astype(bf)
        for b in range(B)
    ]

    def wperm(W, sl):
        # w[p, kd, c] = W[kd*128+p, sl][c]
        return np.ascontiguousarray(
            W[:, sl].reshape(KD, 128, C).transpose(1, 0, 2)
        ).astype(bf)

    in_maps = []
    for c in range(N_CORES):
        b, g = divmod(c, HPC)
        sl = slice(g * C, (g + 1) * C)
        in_maps.append({
            "xT": xTp[b],
            "wq": wperm(Wq, sl),
            "wk": wperm(Wk, sl),
            "wv": wperm(Wv, sl),
            "bq": np.ascontiguousarray(bq[sl]),
            "bk": np.ascontiguousarray(bk[sl]),
            "bv": np.ascontiguousarray(bv[sl]),
        })

    from concourse.bass_utils import run_bass_kernel_spmd

    if _profile:
        _install_ntff_hook()
    res = run_bass_kernel_spmd(nc, in_maps, list(range(N_CORES)),
                               trace=_profile)
    out = np.empty((B, S, D), dtype=np.float32)
    for c in range(N_CORES):
        b, g = divmod(c, HPC)
        out[b, :, g * C : (g + 1) * C] = res.results[c]["y"].astype(np.float32)
    if _profile:
        kernel.last_exec_time_ns = res.exec_time_ns
    return out

